# revision 1
# baseline (speedup 1.0000x reference)
"""Multi-head attention (GAttention) on 8 trn2 NeuronCores.

Reference computation (per batch b):
    q = x @ w_qkv.T            -> [N, 768], heads of 64
    attn = softmax(q k^T / 8)  -> per head [N, M]
    out_h = attn @ v           -> [N, 64]
    out = concat(out_h) @ w_proj.T + b_proj

Sharding: 24 (b, head) units over 8 cores -> each core gets one batch b and
3 heads. Each core computes its heads' attention plus its partial
projection sum [N, 768]; host adds the 4 partials per batch + bias.

Per-core device pipeline:
  1. qproj (f32r): qT_dup[128, N] per head = [wq_h | wq_h]^T x^T; the
     duplicated column block makes rows 64:128 a copy of rows 0:64, which
     feeds the row-packed S^T matmuls.
  2. attention (bf16 operands, f32 PSUM), 6 (head, n-half) units; per key
     m-tile PAIR (2 x 128 keys, PE row groups 0/64 run concurrently):
       S^T = k q^T   -> PSUM [128, 2, 512] per n-chunk (tile A/B)
       expT = exp(0.125 S^T) -> SBUF bf16 (ACT, fused scale)
       AV: av[128, 1024] += v_aug^T expT   (accumulate over all 16 m-tiles)
     v_aug = [v_h | ones*64] so av rows 64:128 hold the softmax denominator.
  3. normalize: outTn (both partition halves) = av[0:64] * recip(av[64:128])
  4. proj (f32r): row-packed n-tile pairs, PSUM accumulates the 3 heads.
"""
import numpy as np
import ml_dtypes
from contextlib import ExitStack

import concourse.bass as bass
import concourse.mybir as mybir
import concourse.tile as tile
from concourse import bacc
from concourse.bass_utils import run_bass_kernel_spmd

B, N, DIM = 2, 2048, 768
H, D = 12, 64
M = 2048
NCORES = 8
HPC = 3            # heads per core
NT = N // 128      # 16 query tiles
MT = M // 128      # 16 key tiles
MP = MT // 2       # 8 key-tile pairs
CT = DIM // 128    # 6 contraction tiles for qproj
NHALF = 1024       # AV psum n-granularity
F32 = mybir.dt.float32
F32R = mybir.dt.float32r
BF16 = mybir.dt.bfloat16

_cached = {}

# dtype config: "fast" = bf16 attention+qproj, "mid" = f32r qproj + bf16 attn,
# "safe" = all f32r
import os
QUALITY = os.environ.get("KQ", "fast")
QP_DT = BF16 if QUALITY == "fast" else F32R
AT_DT = F32R if QUALITY == "safe" else BF16


def build_program():
    nc = bacc.Bacc("TRN2", target_bir_lowering=False, debug=False)
    xT_d = nc.dram_tensor("xT", [DIM, N], QP_DT, kind="ExternalInput")
    wq_d = nc.dram_tensor("wq", [HPC, DIM, 128], QP_DT, kind="ExternalInput")
    kT_d = nc.dram_tensor("kT", [128, HPC, MP, 128], AT_DT,
                          kind="ExternalInput")
    va_d = nc.dram_tensor("va", [HPC, M, 128], AT_DT, kind="ExternalInput")
    wp_d = nc.dram_tensor("wp", [128, HPC, DIM], F32R, kind="ExternalInput")
    out_d = nc.dram_tensor("out", [N, DIM], F32, kind="ExternalOutput")

    with tile.TileContext(nc) as tc, ExitStack() as ctx:
        big = ctx.enter_context(tc.tile_pool(name="big", bufs=1))
        expp = ctx.enter_context(tc.tile_pool(name="expp", bufs=4))
        stg = ctx.enter_context(tc.tile_pool(name="stg", bufs=3))

        # persistent SBUF tensors; DMA order = consumption order: wq and
        # head-0 k/v first (cheap, unblock the first attention unit), then
        # the xT stream that paces qproj, then the rest
        wq_t = big.tile([128, HPC, CT, 128], QP_DT)
        nc.sync.dma_start(
            wq_t[:], wq_d.rearrange("h (c p) d -> p h c d", p=128))
        kT_t = big.tile([128, HPC, MP, 128], AT_DT)
        va_t = big.tile([128, HPC, MT, 128], AT_DT)
        nc.sync.dma_start(kT_t[:, 0, :, :], kT_d[:, 0, :, :])
        nc.sync.dma_start(va_t[:, 0, :, :],
                          va_d[0].rearrange("(t p) e -> p t e", p=128))
        xT_t = [big.tile([128, N], QP_DT, name=f"xT{c}", tag=f"xT{c}")
                for c in range(CT)]
        for c in range(CT):
            nc.sync.dma_start(xT_t[c][:], xT_d[c * 128:(c + 1) * 128, :])
        for h in range(1, HPC):
            nc.sync.dma_start(kT_t[:, h, :, :], kT_d[:, h, :, :])
            nc.sync.dma_start(va_t[:, h, :, :],
                              va_d[h].rearrange("(t p) e -> p t e", p=128))
        wp_t = big.tile([128, HPC, DIM], F32R)
        nc.sync.dma_start(wp_t[:], wp_d[:])
        qT_t = big.tile([128, HPC, N], AT_DT)
        outTn_t = big.tile([128, HPC, N], F32R)

        # phase 1: q projection; wq has the head slice duplicated so rows
        # 64:128 of qT_t replicate rows 0:64
        with tc.tile_pool(name="qp_ps", bufs=1, space="PSUM") as qp_ps:
            for h in range(HPC):
                qp = qp_ps.tile([128, N], F32)
                for c in range(CT):
                    for ch in range(N // 512):
                        nc.tensor.matmul(
                            qp[:, ch * 512:(ch + 1) * 512],
                            wq_t[:, h, c, :],
                            xT_t[c][:, ch * 512:(ch + 1) * 512],
                            start=(c == 0), stop=(c == CT - 1),
                        )
                nc.vector.tensor_copy(qT_t[:, h, :], qp[:])

        # phase 2: attention in 6 (head, n-half) units; m-tile pairs are
        # row-packed on the PE (row groups 0 and 64). The AV matmuls for
        # iteration i are issued AFTER iteration i+1's S^T so the in-order
        # PE queue never stalls behind the EXP wait.
        with tc.tile_pool(name="st_ps", bufs=3, space="PSUM") as st_ps, \
             tc.tile_pool(name="av_ps", bufs=1, space="PSUM") as av_ps:
            av_by_unit = {}

            def _av(pend):
                unit, et, p, cc, first, last = pend[:6]
                av = av_by_unit[unit]
                nc.tensor.matmul(
                    av[:, cc * 512:(cc + 1) * 512],
                    va_t[:, unit[0], 2 * p, :], et[:, 0, :],
                    start=first, stop=False,
                )
                nc.tensor.matmul(
                    av[:, cc * 512:(cc + 1) * 512],
                    va_t[:, unit[0], 2 * p + 1, :], et[:, 1, :],
                    start=False, stop=last,
                )

            def _norm(unit):
                # copy numerator+denominator out fast to release the av slot;
                # reciprocal + normalize then run off the critical path
                h, half = unit
                av = av_by_unit[unit]
                dn = expp.tile([64, NHALF], F32, tag="dn", name="dn")
                nc.vector.tensor_copy(dn[:], av[64:128, :])
                nm = expp.tile([64, NHALF], F32, tag="nm", name="nm")
                nc.vector.tensor_copy(nm[:], av[0:64, :])
                rs = expp.tile([64, NHALF], F32, tag="rs", name="rs")
                nc.vector.reciprocal_approx_fast(rs[:], dn[:])
                nsl = slice(half * NHALF, (half + 1) * NHALF)
                nc.vector.tensor_mul(
                    outTn_t[0:64, h, nsl], nm[:], rs[:])
                nc.vector.tensor_mul(
                    outTn_t[64:128, h, nsl], nm[:], rs[:])

            iters = [(h, half, p, cc)
                     for half in range(N // NHALF) for h in range(HPC)
                     for p in range(MP) for cc in range(NHALF // 512)]
            pend = []
            LAG = 2

            def _flush(limit):
                while len(pend) > limit:
                    pd = pend.pop(0)
                    _av(pd)
                    if pd[6]:
                        _norm(pd[0])

            for h, half, p, cc in iters:
                unit = (h, half)
                if unit not in av_by_unit:
                    av_by_unit[unit] = av_ps.tile(
                        [128, NHALF], F32, tag="av", name="av")
                n0 = half * NHALF + cc * 512
                st = st_ps.tile([128, 2, 512], F32, tag="st", name="st")
                nc.tensor.matmul(
                    st[:, 0, :], kT_t[0:64, h, p, :],
                    qT_t[0:64, h, n0:n0 + 512],
                    start=True, stop=True, tile_position=(0, 0),
                )
                nc.tensor.matmul(
                    st[:, 1, :], kT_t[64:128, h, p, :],
                    qT_t[64:128, h, n0:n0 + 512],
                    start=True, stop=True, tile_position=(64, 0),
                )
                _flush(LAG - 1)
                et = expp.tile([128, 2, 512], AT_DT, tag="et", name="et")
                nc.scalar.activation(
                    et[:], st[:], mybir.ActivationFunctionType.Exp,
                    scale=float(D) ** -0.5,
                )
                pend.append((unit, et, p, cc, p == 0, p == MP - 1,
                             p == MP - 1 and cc == NHALF // 512 - 1))
            _flush(0)

        # phase 3: projection, row-packed n-tile pairs, PSUM accumulates
        # the 3 heads
        with tc.tile_pool(name="pj_ps", bufs=2, space="PSUM") as pj_ps:
            for nj in range(NT // 2):
                ppa = pj_ps.tile([128, 2, 512], F32, tag="ppa")
                ppb = pj_ps.tile([128, 2, 512], F32, tag="ppb")
                na = 2 * nj * 128
                nb = (2 * nj + 1) * 128
                for h in range(HPC):
                    for oc in range(2):
                        osl = slice(oc * 384, (oc + 1) * 384)
                        nc.tensor.matmul(
                            ppa[:, oc, 0:384],
                            outTn_t[0:64, h, na:na + 128],
                            wp_t[0:64, h, osl],
                            start=(h == 0), stop=(h == HPC - 1),
                            tile_position=(0, 0),
                        )
                        nc.tensor.matmul(
                            ppb[:, oc, 0:384],
                            outTn_t[64:128, h, nb:nb + 128],
                            wp_t[64:128, h, osl],
                            start=(h == 0), stop=(h == HPC - 1),
                            tile_position=(64, 0),
                        )
                for which, pp, nn in ((0, ppa, na), (1, ppb, nb)):
                    os_t = stg.tile([128, DIM], F32, tag="os", name="os")
                    if which == 0:
                        nc.vector.tensor_copy(os_t[:, 0:384], pp[:, 0, 0:384])
                        nc.vector.tensor_copy(os_t[:, 384:768],
                                              pp[:, 1, 0:384])
                    else:
                        nc.scalar.copy(os_t[:, 0:384], pp[:, 0, 0:384])
                        nc.scalar.copy(os_t[:, 384:768], pp[:, 1, 0:384])
                    nc.sync.dma_start(out_d[nn:nn + 128, :], os_t[:])

    nc.compile()
    return nc


def build_in_maps(x, k, v, w_qkv, w_proj):
    x = np.asarray(x, dtype=np.float32)
    k = np.asarray(k, dtype=np.float32)
    v = np.asarray(v, dtype=np.float32)
    wqT = np.ascontiguousarray(np.asarray(w_qkv, np.float32).T)   # [C, 768]
    wpT = np.ascontiguousarray(np.asarray(w_proj, np.float32).T)  # [768, 768]

    in_maps = []
    for core in range(NCORES):
        b = core // 4
        hs = [3 * (core % 4) + i for i in range(HPC)]
        qp_np = ml_dtypes.bfloat16 if QUALITY == "fast" else np.float32
        at_np = np.float32 if QUALITY == "safe" else ml_dtypes.bfloat16
        xT = np.ascontiguousarray(x[b].T.astype(qp_np))
        # duplicated head slice -> qT rows 64:128 == rows 0:64
        wq = np.stack([
            np.concatenate([wqT[:, 64 * h:64 * (h + 1)]] * 2, axis=1)
            for h in hs]).astype(qp_np)                          # [3, DIM, 128]
        # kT layout [128, HPC, MP, 128]: rows 0:64 = head-dim of even m-tile,
        # rows 64:128 = head-dim of odd m-tile of each pair
        kb = k[b, hs].astype(at_np)                              # [3, M, D]
        kT = np.empty((128, HPC, MP, 128), dtype=at_np)
        for hi in range(HPC):
            for p in range(MP):
                kT[0:64, hi, p, :] = kb[hi, 256 * p:256 * p + 128, :].T
                kT[64:128, hi, p, :] = kb[hi, 256 * p + 128:256 * p + 256, :].T
        va = np.ones((HPC, M, 128), dtype=at_np)
        va[:, :, :D] = v[b, hs].astype(at_np)       # [3, M, 128]
        # wp duplicated on both partition halves for row-packed proj
        wp = np.empty((128, HPC, DIM), dtype=np.float32)
        for hi, h in enumerate(hs):
            wp[0:64, hi, :] = wpT[64 * h:64 * (h + 1), :]
            wp[64:128, hi, :] = wpT[64 * h:64 * (h + 1), :]
        in_maps.append({"xT": xT, "wq": wq,
                        "kT": np.ascontiguousarray(kT),
                        "va": np.ascontiguousarray(va),
                        "wp": np.ascontiguousarray(wp)})
    return in_maps


def kernel(x, k, v, w_qkv, w_proj, b_proj):
    b_proj = np.asarray(b_proj, dtype=np.float32)

    if "nc" not in _cached:
        _cached["nc"] = build_program()
    nc = _cached["nc"]

    in_maps = build_in_maps(x, k, v, w_qkv, w_proj)
    res = run_bass_kernel_spmd(nc, in_maps, core_ids=list(range(NCORES)))

    out = np.empty((B, N, DIM), dtype=np.float32)
    for b in range(B):
        acc = np.zeros((N, DIM), dtype=np.float64)
        for core in range(4 * b, 4 * b + 4):
            acc += res.results[core]["out"]
        out[b] = (acc + b_proj).astype(np.float32)
    return out



# revision 11
# speedup vs baseline: 1.1974x; 1.1974x over previous
"""Multi-head attention (GAttention) on 8 trn2 NeuronCores.

Reference computation (per batch b):
    q = x @ w_qkv.T            -> [N, 768], heads of 64
    attn = softmax(q k^T / 8)  -> per head [N, M]
    out_h = attn @ v           -> [N, 64]
    out = concat(out_h) @ w_proj.T + b_proj

Sharding: 24 (b, head) units over 8 cores -> each core gets one batch b and
3 heads. Each core computes its heads' attention plus its partial
projection sum [N, 768]; host adds the 4 partials per batch + bias.

The kernel is ScalarE-bound: softmax exp is ACT-only at 1 elem/lane/cycle
(1.2 GHz), 12.6M elems/core ~= 96 us. Everything else is scheduled around
keeping the ACT exp stream dense from ~9 us on:
  1. qproj (bf16) paced by the xT DMA stream, n-half outer so head-0
     queries finish early; wq has the head slice duplicated so qT rows
     64:128 copy rows 0:64 (feeds the row-packed S^T matmuls).
  2. attention in 12 (head, n-quarter) units; per key m-tile PAIR:
       S^T = k q^T   (PE row groups 0/64 concurrently) -> PSUM [128,2,512]
       expT = exp(0.125 S^T) -> SBUF bf16 (ACT, fused scale)
       AV: av[128,512] += va^T expT  (va = [v | ones]; rows 64:128 of av
           hold the softmax denominator), issued LAG iterations behind.
  3. normalize: outTn[0:64] = av[0:64] * recip_approx(av[64:128])
  4. proj (bf16): per 128-row n-tile, 3 heads accumulate in PSUM; output
     DMA'd straight from PSUM to DRAM f32. Proj tiles are interleaved into
     the following attention unit so only the last quarter is tail work.
"""
import numpy as np
import ml_dtypes
from contextlib import ExitStack

import concourse.bass as bass
import concourse.mybir as mybir
import concourse.tile as tile
from concourse import bacc
from concourse.bass_utils import run_bass_kernel_spmd

B, N, DIM = 2, 2048, 768
H, D = 12, 64
M = 2048
NCORES = 8
HPC = 3            # heads per core
NT = N // 128      # 16 query tiles
MT = M // 128      # 16 key tiles
MP = MT // 2       # 8 key-tile pairs
CT = DIM // 128    # 6 contraction tiles for qproj
QN = 512           # attention-unit query granularity (av psum = 1 bank)
NQ = N // QN       # 4 quarters
F32 = mybir.dt.float32
BF16 = mybir.dt.bfloat16

_cached = {}
DEBUG_TAPS = False


def build_program():
    nc = bacc.Bacc("TRN2", target_bir_lowering=False, debug=False)
    xT_d = nc.dram_tensor("xT", [DIM, N], BF16, kind="ExternalInput")
    wq_d = nc.dram_tensor("wq", [128, HPC, CT, 128], BF16,
                          kind="ExternalInput")
    kT_d = nc.dram_tensor("kT", [128, HPC, MP, 128], BF16,
                          kind="ExternalInput")
    va_d = nc.dram_tensor("va", [128, HPC, MT, 128], BF16,
                          kind="ExternalInput")
    wp_d = nc.dram_tensor("wp", [64, HPC, DIM], BF16, kind="ExternalInput")
    out_d = nc.dram_tensor("out", [N, DIM], F32, kind="ExternalOutput")
    if DEBUG_TAPS:
        qTdump_d = nc.dram_tensor("qTdump", [128, HPC, N], BF16,
                                  kind="ExternalOutput")
        otdump_d = nc.dram_tensor("otdump", [64, HPC, N], BF16,
                                  kind="ExternalOutput")

    with tile.TileContext(nc) as tc, ExitStack() as ctx:
        big = ctx.enter_context(tc.tile_pool(name="big", bufs=1))
        expp = ctx.enter_context(tc.tile_pool(name="expp", bufs=4))
        nrm = ctx.enter_context(tc.tile_pool(name="nrm", bufs=3))
        stg = ctx.enter_context(tc.tile_pool(name="stg", bufs=3))

        # ACT table warmup: a tiny exp at t~0 so the ~2.7us table load is
        # off the critical path of the first real exp
        wu = big.tile([128, 8], F32)
        nc.gpsimd.memset(wu[:], 0.0)
        wu2 = big.tile([128, 8], F32)
        nc.scalar.activation(wu2[:], wu[:], mybir.ActivationFunctionType.Exp)

        # persistent SBUF tensors; DMA order = consumption order: wq, then
        # per-quarter xT slices interleaved with per-head k/v
        wq_t = big.tile([128, HPC, CT, 128], BF16)
        nc.sync.dma_start(wq_t[:], wq_d[:])
        xT_t = [big.tile([128, N], BF16, name=f"xT{c}", tag=f"xT{c}")
                for c in range(CT)]
        kT_t = big.tile([128, HPC, MP, 128], BF16)
        va_t = big.tile([128, HPC, MT, 128], BF16)
        for q in range(NQ):
            for c in range(CT):
                nc.sync.dma_start(xT_t[c][:, q * QN:(q + 1) * QN],
                                  xT_d[c * 128:(c + 1) * 128,
                                       q * QN:(q + 1) * QN])
            if q < HPC:
                nc.sync.dma_start(kT_t[:, q], kT_d[:, q])
                nc.sync.dma_start(va_t[:, q], va_d[:, q])
        wp_t = big.tile([64, HPC, DIM], BF16)
        nc.sync.dma_start(wp_t[:], wp_d[:])

        qT_t = big.tile([128, HPC, N], BF16)
        outTn_t = big.tile([64, HPC, N], BF16)

        # single instruction stream: per quarter, qproj (into the st psum
        # pool) then the 3 attention units; proj tiles of the previous
        # quarter interleave one per iteration. PSUM: st 2x2 banks +
        # av 2x1 + pj 2x1 + 2 spare.
        with tc.tile_pool(name="st_ps", bufs=2, space="PSUM") as st_ps, \
             tc.tile_pool(name="av_ps", bufs=2, space="PSUM") as av_ps, \
             tc.tile_pool(name="pj_ps", bufs=2, space="PSUM") as pj_ps:
            av_by_unit = {}
            pend = []
            proj_todo = []
            LAG = 2

            def _av(pd):
                (h, q), et, p, first, last = pd
                av = av_by_unit[(h, q)]
                nc.tensor.matmul(av[:], va_t[:, h, 2 * p], et[:, 0],
                                 start=first, stop=False)
                nc.tensor.matmul(av[:], va_t[:, h, 2 * p + 1], et[:, 1],
                                 start=False, stop=last)

            def _norm(unit):
                # copy av halves to base-partition-0 SBUF tiles first: the
                # custom-DVE recip misbehaves on HW when its input AP sits at
                # a partition offset (sim-only correct)
                h, q = unit
                av = av_by_unit.pop(unit)
                dn = nrm.tile([64, QN], F32, tag="dn", name="dn")
                nc.vector.tensor_copy(dn[:], av[64:128, :])
                nm = nrm.tile([64, QN], F32, tag="nm", name="nm")
                nc.vector.tensor_copy(nm[:], av[0:64, :])
                rs = nrm.tile([64, QN], F32, tag="rs", name="rs")
                nc.vector.reciprocal_approx_fast(rs[:], dn[:])
                nc.vector.tensor_mul(
                    outTn_t[0:64, h, q * QN:(q + 1) * QN], nm[:], rs[:])

            def _qproj(q, h):
                # q projection for one (quarter, head), accumulated in an
                # st-pool buffer; wq's duplicated head slice makes qT rows
                # 64:128 a copy of rows 0:64
                qp = st_ps.tile([128, 2, 512], F32, tag="st", name="qp")
                for c in range(CT):
                    nc.tensor.matmul(
                        qp[:, 0], wq_t[:, h, c],
                        xT_t[c][:, q * QN:(q + 1) * QN],
                        start=(c == 0), stop=(c == CT - 1),
                    )
                nc.vector.tensor_copy(
                    qT_t[:, h, q * QN:(q + 1) * QN], qp[:, 0])

            def _proj_half(q, j, oc):
                # one 128-row n-tile x 384 out-cols; 3 heads accumulate
                nn = (q * 4 + j) * 128
                pp = pj_ps.tile([128, 512], F32, tag="pp", name="pp")
                for h in range(HPC):
                    nc.tensor.matmul(
                        pp[:, 0:384],
                        outTn_t[0:64, h, nn:nn + 128],
                        wp_t[:, h, oc * 384:(oc + 1) * 384],
                        start=(h == 0), stop=(h == HPC - 1),
                        tile_position=(0, 0),
                    )
                os_t = stg.tile([128, 384], F32, tag="os", name="os")
                nc.vector.tensor_copy(os_t[:], pp[:, 0:384])
                nc.sync.dma_start(
                    out_d[nn:nn + 128, oc * 384:(oc + 1) * 384], os_t[:])

            def _flush(limit):
                while len(pend) > limit:
                    pd = pend.pop(0)
                    _av(pd)
                    if pd[4]:
                        _norm(pd[0])
                        h, q = pd[0]
                        if h == HPC - 1:
                            proj_todo.extend(
                                (q, j, oc) for j in range(4) for oc in range(2))

            for h in range(HPC):
                _qproj(0, h)
            iters = [(h, q, p)
                     for q in range(NQ) for h in range(HPC) for p in range(MP)]
            for h, q, p in iters:
                unit = (h, q)
                if unit not in av_by_unit:
                    av_by_unit[unit] = av_ps.tile(
                        [128, QN], F32, tag="av", name="av")
                n0 = q * QN
                st = st_ps.tile([128, 2, 512], F32, tag="st", name="st")
                nc.tensor.matmul(
                    st[:, 0], kT_t[0:64, h, p], qT_t[0:64, h, n0:n0 + QN],
                    start=True, stop=True, tile_position=(0, 0),
                )
                nc.tensor.matmul(
                    st[:, 1], kT_t[64:128, h, p], qT_t[64:128, h, n0:n0 + QN],
                    start=True, stop=True, tile_position=(64, 0),
                )
                _flush(LAG - 1)
                et = expp.tile([128, 2, 512], BF16, tag="et", name="et")
                nc.scalar.activation(
                    et[:], st[:], mybir.ActivationFunctionType.Exp,
                    scale=float(D) ** -0.5,
                )
                pend.append((unit, et, p, p == 0, p == MP - 1))
                if p == MP - 1 and q + 1 < NQ:
                    _qproj(q + 1, h)
                if proj_todo:
                    _proj_half(*proj_todo.pop(0))
            _flush(0)
            while proj_todo:
                _proj_half(*proj_todo.pop(0))
            if DEBUG_TAPS:
                nc.sync.dma_start(qTdump_d[:], qT_t[:])
                nc.sync.dma_start(otdump_d[:], outTn_t[:])

    nc.compile()
    return nc


def build_in_maps(x, k, v, w_qkv, w_proj):
    x = np.asarray(x, dtype=np.float32)
    k = np.asarray(k, dtype=np.float32)
    v = np.asarray(v, dtype=np.float32)
    wqT = np.ascontiguousarray(np.asarray(w_qkv, np.float32).T)   # [C, 768]
    wpT = np.ascontiguousarray(np.asarray(w_proj, np.float32).T)  # [768, 768]
    bf = ml_dtypes.bfloat16

    in_maps = []
    for core in range(NCORES):
        b = core // 4
        hs = [3 * (core % 4) + i for i in range(HPC)]
        xT = np.ascontiguousarray(x[b].T.astype(bf))
        # wq [128, HPC, CT, 128]: head slice duplicated -> qT rows
        # 64:128 == rows 0:64
        wq = np.empty((128, HPC, CT, 128), dtype=bf)
        for hi, h in enumerate(hs):
            blk = wqT[:, 64 * h:64 * (h + 1)]
            dup = np.concatenate([blk, blk], axis=1)        # [768, 128]
            wq[:, hi] = dup.reshape(CT, 128, 128).transpose(1, 0, 2).astype(bf)
        # kT [128, HPC, MP, 128]: rows 0:64 = head-dim of even m-tile,
        # rows 64:128 = head-dim of odd m-tile of each pair
        kb = k[b, hs].astype(bf)                            # [3, M, D]
        kT = np.empty((128, HPC, MP, 128), dtype=bf)
        for hi in range(HPC):
            for p in range(MP):
                kT[0:64, hi, p, :] = kb[hi, 256 * p:256 * p + 128, :].T
                kT[64:128, hi, p, :] = kb[hi, 256 * p + 128:256 * p + 256, :].T
        # va [128, HPC, MT, 128]: [v | ones]; partition = key-within-tile
        va = np.ones((128, HPC, MT, 128), dtype=bf)
        va[:, :, :, :D] = (
            v[b, hs].reshape(HPC, MT, 128, D).transpose(2, 0, 1, 3).astype(bf))
        # wp [64, HPC, DIM]
        wp = np.empty((64, HPC, DIM), dtype=bf)
        for hi, h in enumerate(hs):
            wp[:, hi, :] = wpT[64 * h:64 * (h + 1), :].astype(bf)
        in_maps.append({"xT": xT, "wq": wq,
                        "kT": np.ascontiguousarray(kT),
                        "va": np.ascontiguousarray(va),
                        "wp": np.ascontiguousarray(wp)})
    return in_maps


def kernel(x, k, v, w_qkv, w_proj, b_proj):
    b_proj = np.asarray(b_proj, dtype=np.float32)

    if "nc" not in _cached:
        _cached["nc"] = build_program()
    nc = _cached["nc"]

    in_maps = build_in_maps(x, k, v, w_qkv, w_proj)
    res = run_bass_kernel_spmd(nc, in_maps, core_ids=list(range(NCORES)))

    out = np.empty((B, N, DIM), dtype=np.float32)
    for b in range(B):
        acc = np.zeros((N, DIM), dtype=np.float64)
        for core in range(4 * b, 4 * b + 4):
            acc += res.results[core]["out"]
        out[b] = (acc + b_proj).astype(np.float32)
    return out


# revision 24
# speedup vs baseline: 1.2575x; 1.0503x over previous
"""Multi-head attention (GAttention) on 8 trn2 NeuronCores.

Reference computation (per batch b):
    q = x @ w_qkv.T            -> [N, 768], heads of 64
    attn = softmax(q k^T / 8)  -> per head [N, M]
    out_h = attn @ v           -> [N, 64]
    out = concat(out_h) @ w_proj.T + b_proj

Sharding: 24 (b, head) units over 8 cores -> each core gets one batch b and
3 heads. Each core computes its heads' attention plus its partial
projection sum [N, 768]; host adds the 4 partials per batch + bias.

The kernel is ScalarE-bound: softmax exp is ACT-only at 1 elem/lane/cycle
(1.2 GHz), 12.6M elems/core ~= 96 us. Everything else is scheduled around
keeping the ACT exp stream dense from ~9 us on:
  1. qproj (bf16) paced by the xT DMA stream, n-half outer so head-0
     queries finish early; wq has the head slice duplicated so qT rows
     64:128 copy rows 0:64 (feeds the row-packed S^T matmuls).
  2. attention in 12 (head, n-quarter) units; per key m-tile PAIR:
       S^T = k q^T   (PE row groups 0/64 concurrently) -> PSUM [128,2,512]
       expT = exp(0.125 S^T) -> SBUF bf16 (ACT, fused scale)
       AV: av[128,512] += va^T expT  (va = [v | ones]; rows 64:128 of av
           hold the softmax denominator), issued LAG iterations behind.
  3. normalize: outTn[0:64] = av[0:64] * recip_approx(av[64:128])
  4. proj (bf16): per 128-row n-tile, 3 heads accumulate in PSUM; output
     DMA'd straight from PSUM to DRAM f32. Proj tiles are interleaved into
     the following attention unit so only the last quarter is tail work.
"""
import numpy as np
import ml_dtypes
from contextlib import ExitStack

import concourse.bass as bass
import concourse.mybir as mybir
import concourse.tile as tile
from concourse import bacc
from concourse.bass_utils import run_bass_kernel_spmd

B, N, DIM = 2, 2048, 768
H, D = 12, 64
M = 2048
NCORES = 8
HPC = 3            # heads per core
NT = N // 128      # 16 query tiles
MT = M // 128      # 16 key tiles
MP = MT // 2       # 8 key-tile pairs
CT = DIM // 128    # 6 contraction tiles for qproj
QN = 512           # attention-unit query granularity (av psum = 1 bank)
NQ = N // QN       # 4 quarters
F32 = mybir.dt.float32
BF16 = mybir.dt.bfloat16
I16 = mybir.dt.int16

# Schraudolph fast-exp constants for the DVE offload path, in bf16
# bit-space: i16 = convert(s * A + B); bitcast(i16) as bf16 ~= exp(0.125*s),
# max rel err ~3%. A = 2^7 * 0.125 * log2(e); B = (127 - 0.0436) * 2^7
# (host-tuned minimax shift).
EXP_A = 23.083120654232846
EXP_B = 16250.4192
# m-tile pairs whose exp runs on the DVE instead of ScalarE
DVE_PAIRS = {2, 5}

_cached = {}
DEBUG_TAPS = False


def build_program():
    nc = bacc.Bacc("TRN2", target_bir_lowering=False, debug=False)
    xT_d = nc.dram_tensor("xT", [DIM, N], BF16, kind="ExternalInput")
    wq_d = nc.dram_tensor("wq", [128, HPC, CT, 128], BF16,
                          kind="ExternalInput")
    kT_d = nc.dram_tensor("kT", [128, HPC, MP, 128], BF16,
                          kind="ExternalInput")
    va_d = nc.dram_tensor("va", [128, HPC, MT, 128], BF16,
                          kind="ExternalInput")
    wp01_d = nc.dram_tensor("wp01", [128, DIM], BF16, kind="ExternalInput")
    wp2_d = nc.dram_tensor("wp2", [64, DIM], BF16, kind="ExternalInput")
    out_d = nc.dram_tensor("out", [N, DIM], F32, kind="ExternalOutput")
    if DEBUG_TAPS:
        qTdump_d = nc.dram_tensor("qTdump", [128, HPC, N], BF16,
                                  kind="ExternalOutput")
        otdump_d = nc.dram_tensor("otdump", [128, N], BF16,
                                  kind="ExternalOutput")
        ot2dump_d = nc.dram_tensor("ot2dump", [64, N], BF16,
                                   kind="ExternalOutput")

    with tile.TileContext(nc) as tc, ExitStack() as ctx:
        big = ctx.enter_context(tc.tile_pool(name="big", bufs=1))
        expp = ctx.enter_context(tc.tile_pool(name="expp", bufs=4))
        expi = ctx.enter_context(tc.tile_pool(name="expi", bufs=3))
        nrm = ctx.enter_context(tc.tile_pool(name="nrm", bufs=3))
        stg = ctx.enter_context(tc.tile_pool(name="stg", bufs=3))

        # ACT table warmup: a tiny exp at t~0 so the ~2.7us table load is
        # off the critical path of the first real exp
        wu = big.tile([128, 8], F32)
        nc.gpsimd.memset(wu[:], 0.0)
        wu2 = big.tile([128, 8], F32)
        nc.scalar.activation(wu2[:], wu[:], mybir.ActivationFunctionType.Exp)

        # persistent SBUF tensors; DMA order = consumption order: wq and
        # xT quarter 0 pace qproj(q0), then all k/v (units of quarter 0 run
        # through all 3 heads), then the later xT quarters and wp.
        # va_r (f32 copy for the DVE-exp AV path) is converted on the idle
        # gpsimd engine as each va head lands.
        wq_t = big.tile([128, HPC, CT, 128], BF16)
        nc.sync.dma_start(wq_t[:], wq_d[:])
        xT_t = [big.tile([128, N], BF16, name=f"xT{c}", tag=f"xT{c}")
                for c in range(CT)]
        kT_t = big.tile([128, HPC, MP, 128], BF16)
        va_t = big.tile([128, HPC, MT, 128], BF16)

        def _dma_xq(q):
            for c in range(CT):
                nc.sync.dma_start(xT_t[c][:, q * QN:(q + 1) * QN],
                                  xT_d[c * 128:(c + 1) * 128,
                                       q * QN:(q + 1) * QN])

        def _dma_kv(h):
            nc.sync.dma_start(kT_t[:, h], kT_d[:, h])
            nc.sync.dma_start(va_t[:, h], va_d[:, h])

        _dma_xq(0)
        _dma_kv(0)
        _dma_kv(1)
        _dma_xq(1)
        _dma_kv(2)
        wp01_t = big.tile([128, DIM], BF16)
        nc.sync.dma_start(wp01_t[:], wp01_d[:])
        wp2_t = big.tile([64, DIM], BF16)
        nc.sync.dma_start(wp2_t[:], wp2_d[:])
        _dma_xq(2)
        _dma_xq(3)

        qT_t = big.tile([128, HPC, N], BF16)
        # proj contraction operands: heads 0|1 stacked on the partition dim,
        # head 2 separate
        outTn01_t = big.tile([128, N], BF16)
        outTn2_t = big.tile([64, N], BF16)

        # single instruction stream: per quarter, qproj (into the st psum
        # pool) then the 3 attention units; proj tiles of the previous
        # quarter interleave one per iteration. PSUM: st 2x2 banks +
        # av 2x1 + pj 2x1 + 2 spare.
        with tc.tile_pool(name="st_ps", bufs=2, space="PSUM") as st_ps, \
             tc.tile_pool(name="av_ps", bufs=2, space="PSUM") as av_ps, \
             tc.tile_pool(name="pj_ps", bufs=2, space="PSUM") as pj_ps:
            av_by_unit = {}
            pend = []
            proj_todo = []
            LAG = 2

            def _av(pd):
                (h, q), et0, et1, p, first, last = pd
                av = av_by_unit[(h, q)]
                nc.tensor.matmul(av[:], va_t[:, h, 2 * p], et0,
                                 start=first, stop=False)
                nc.tensor.matmul(av[:], va_t[:, h, 2 * p + 1], et1,
                                 start=False, stop=last)

            def _norm(unit):
                # denominator copied to a base-partition-0 SBUF tile first:
                # the custom-DVE recip misbehaves on HW when its input AP
                # sits at a partition offset (sim-only correct)
                h, q = unit
                av = av_by_unit.pop(unit)
                nsl = slice(q * QN, (q + 1) * QN)
                dn = nrm.tile([64, QN], F32, tag="dn", name="dn")
                nc.vector.tensor_copy(dn[:], av[64:128, :])
                rs = nrm.tile([64, QN], F32, tag="rs", name="rs")
                nc.vector.reciprocal_approx_fast(rs[:], dn[:])
                if h == 0:
                    dst = outTn01_t[0:64, nsl]
                elif h == 1:
                    dst = outTn01_t[64:128, nsl]
                else:
                    dst = outTn2_t[:, nsl]
                nc.vector.tensor_mul(dst, av[0:64, :], rs[:])

            qp_by = {}

            def _qproj_part(q, h, part):
                # q projection for one (quarter, head), 2 c-tiles per call so
                # the PE bubble it injects into the attention stream stays
                # small; accumulates in a pj-pool buffer. wq's duplicated
                # head slice makes qT rows 64:128 a copy of rows 0:64
                if part == 0:
                    qp_by[(q, h)] = pj_ps.tile([128, 512], F32,
                                               tag="pp", name="qp")
                qp = qp_by[(q, h)]
                for c in (2 * part, 2 * part + 1):
                    nc.tensor.matmul(
                        qp[:], wq_t[:, h, c],
                        xT_t[c][:, q * QN:(q + 1) * QN],
                        start=(c == 0), stop=(c == CT - 1),
                    )
                if part == 2:
                    nc.vector.tensor_copy(
                        qT_t[:, h, q * QN:(q + 1) * QN], qp[:])
                    del qp_by[(q, h)]

            def _proj_half(q, j, oc):
                # one 128-row n-tile x 384 out-cols; heads 0|1 via a single
                # 128-deep contraction, head 2 accumulated on top
                nn = (q * 4 + j) * 128
                osl = slice(oc * 384, (oc + 1) * 384)
                pp = pj_ps.tile([128, 512], F32, tag="pp", name="pp")
                nc.tensor.matmul(pp[:, 0:384], outTn01_t[:, nn:nn + 128],
                                 wp01_t[:, osl], start=True, stop=False)
                nc.tensor.matmul(pp[:, 0:384], outTn2_t[:, nn:nn + 128],
                                 wp2_t[:, osl], start=False, stop=True,
                                 tile_position=(0, 0))
                os_t = stg.tile([128, 384], F32, tag="os", name="os")
                nc.vector.tensor_copy(os_t[:], pp[:, 0:384])
                nc.sync.dma_start(out_d[nn:nn + 128, osl], os_t[:])

            def _flush(limit):
                while len(pend) > limit:
                    pd = pend.pop(0)
                    _av(pd)
                    if pd[5]:
                        _norm(pd[0])
                        h, q = pd[0]
                        if h == HPC - 1:
                            proj_todo.extend(
                                (q, j, oc) for j in range(4) for oc in range(2))

            for h in range(HPC):
                for part in range(3):
                    _qproj_part(0, h, part)
            iters = [(h, q, p)
                     for q in range(NQ) for h in range(HPC) for p in range(MP)]
            for h, q, p in iters:
                unit = (h, q)
                if unit not in av_by_unit:
                    av_by_unit[unit] = av_ps.tile(
                        [128, QN], F32, tag="av", name="av")
                n0 = q * QN
                st = st_ps.tile([128, 2, 512], F32, tag="st", name="st")
                nc.tensor.matmul(
                    st[:, 0], kT_t[0:64, h, p], qT_t[0:64, h, n0:n0 + QN],
                    start=True, stop=True, tile_position=(0, 0),
                )
                nc.tensor.matmul(
                    st[:, 1], kT_t[64:128, h, p], qT_t[64:128, h, n0:n0 + QN],
                    start=True, stop=True, tile_position=(64, 0),
                )
                _flush(LAG - 1)
                if p in DVE_PAIRS:
                    eti = expi.tile([128, 2, 512], I16, tag="eti", name="eti")
                    nc.vector.tensor_scalar(
                        eti[:], st[:], EXP_A, EXP_B,
                        mybir.AluOpType.mult, mybir.AluOpType.add)
                    et0 = eti[:, 0].bitcast(BF16)
                    et1 = eti[:, 1].bitcast(BF16)
                else:
                    et = expp.tile([128, 2, 512], BF16, tag="et", name="et")
                    nc.scalar.activation(
                        et[:], st[:], mybir.ActivationFunctionType.Exp,
                        scale=float(D) ** -0.5,
                    )
                    et0, et1 = et[:, 0], et[:, 1]
                pend.append((unit, et0, et1, p, p == 0, p == MP - 1))
                if q + 1 < NQ and p >= MP - 3:
                    _qproj_part(q + 1, h, p - (MP - 3))
                if proj_todo:
                    _proj_half(*proj_todo.pop(0))
            _flush(0)
            while proj_todo:
                _proj_half(*proj_todo.pop(0))
            if DEBUG_TAPS:
                nc.sync.dma_start(qTdump_d[:], qT_t[:])
                nc.sync.dma_start(otdump_d[:], outTn01_t[:])
                nc.sync.dma_start(ot2dump_d[:], outTn2_t[:])

    nc.compile()
    return nc


def build_in_maps(x, k, v, w_qkv, w_proj):
    x = np.asarray(x, dtype=np.float32)
    k = np.asarray(k, dtype=np.float32)
    v = np.asarray(v, dtype=np.float32)
    wqT = np.ascontiguousarray(np.asarray(w_qkv, np.float32).T)   # [C, 768]
    wpT = np.ascontiguousarray(np.asarray(w_proj, np.float32).T)  # [768, 768]
    bf = ml_dtypes.bfloat16

    in_maps = []
    for core in range(NCORES):
        b = core // 4
        hs = [3 * (core % 4) + i for i in range(HPC)]
        xT = np.ascontiguousarray(x[b].T.astype(bf))
        # wq [128, HPC, CT, 128]: head slice duplicated -> qT rows
        # 64:128 == rows 0:64
        wq = np.empty((128, HPC, CT, 128), dtype=bf)
        for hi, h in enumerate(hs):
            blk = wqT[:, 64 * h:64 * (h + 1)]
            dup = np.concatenate([blk, blk], axis=1)        # [768, 128]
            wq[:, hi] = dup.reshape(CT, 128, 128).transpose(1, 0, 2).astype(bf)
        # kT [128, HPC, MP, 128]: rows 0:64 = head-dim of even m-tile,
        # rows 64:128 = head-dim of odd m-tile of each pair
        kb = k[b, hs].astype(bf)                            # [3, M, D]
        kT = np.empty((128, HPC, MP, 128), dtype=bf)
        for hi in range(HPC):
            for p in range(MP):
                kT[0:64, hi, p, :] = kb[hi, 256 * p:256 * p + 128, :].T
                kT[64:128, hi, p, :] = kb[hi, 256 * p + 128:256 * p + 256, :].T
        # va [128, HPC, MT, 128]: [v | ones]; partition = key-within-tile
        va = np.ones((128, HPC, MT, 128), dtype=bf)
        va[:, :, :, :D] = (
            v[b, hs].reshape(HPC, MT, 128, D).transpose(2, 0, 1, 3).astype(bf))
        # wp01 [128, DIM]: heads 0|1 stacked on partitions; wp2 [64, DIM]
        wp01 = np.empty((128, DIM), dtype=bf)
        wp01[0:64] = wpT[64 * hs[0]:64 * hs[0] + 64, :].astype(bf)
        wp01[64:128] = wpT[64 * hs[1]:64 * hs[1] + 64, :].astype(bf)
        wp2 = np.ascontiguousarray(
            wpT[64 * hs[2]:64 * hs[2] + 64, :].astype(bf))
        in_maps.append({"xT": xT, "wq": wq,
                        "kT": np.ascontiguousarray(kT),
                        "va": np.ascontiguousarray(va),
                        "wp01": wp01, "wp2": wp2})
    return in_maps


def kernel(x, k, v, w_qkv, w_proj, b_proj):
    b_proj = np.asarray(b_proj, dtype=np.float32)

    if "nc" not in _cached:
        _cached["nc"] = build_program()
    nc = _cached["nc"]

    in_maps = build_in_maps(x, k, v, w_qkv, w_proj)
    res = run_bass_kernel_spmd(nc, in_maps, core_ids=list(range(NCORES)))

    out = np.empty((B, N, DIM), dtype=np.float32)
    for b in range(B):
        acc = np.zeros((N, DIM), dtype=np.float64)
        for core in range(4 * b, 4 * b + 4):
            acc += res.results[core]["out"]
        out[b] = (acc + b_proj).astype(np.float32)
    return out


# revision 34
# speedup vs baseline: 1.2751x; 1.0140x over previous
"""Multi-head attention (GAttention) on 8 trn2 NeuronCores.

Reference computation (per batch b):
    q = x @ w_qkv.T            -> [N, 768], heads of 64
    attn = softmax(q k^T / 8)  -> per head [N, M]
    out_h = attn @ v           -> [N, 64]
    out = concat(out_h) @ w_proj.T + b_proj

Sharding: 24 (b, head) units over 8 cores -> each core gets one batch b and
3 heads. Each core computes its heads' attention plus its partial
projection sum [N, 768]; host adds the 4 partials per batch + bias.

The kernel is ScalarE-bound: softmax exp is ACT-only at 1 elem/lane/cycle
(1.2 GHz), 12.6M elems/core ~= 96 us. Everything else is scheduled around
keeping the ACT exp stream dense from ~9 us on:
  1. qproj (bf16) paced by the xT DMA stream, n-half outer so head-0
     queries finish early; wq has the head slice duplicated so qT rows
     64:128 copy rows 0:64 (feeds the row-packed S^T matmuls).
  2. attention in 12 (head, n-quarter) units; per key m-tile PAIR:
       S^T = k q^T   (PE row groups 0/64 concurrently) -> PSUM [128,2,512]
       expT = exp(0.125 S^T) -> SBUF bf16 (ACT, fused scale)
       AV: av[128,512] += va^T expT  (va = [v | ones]; rows 64:128 of av
           hold the softmax denominator), issued LAG iterations behind.
  3. normalize: outTn[0:64] = av[0:64] * recip_approx(av[64:128])
  4. proj (bf16): per 128-row n-tile, 3 heads accumulate in PSUM; output
     DMA'd straight from PSUM to DRAM f32. Proj tiles are interleaved into
     the following attention unit so only the last quarter is tail work.
"""
import numpy as np
import ml_dtypes
from contextlib import ExitStack

import concourse.bass as bass
import concourse.mybir as mybir
import concourse.tile as tile
from concourse import bacc
from concourse.bass_utils import run_bass_kernel_spmd

B, N, DIM = 2, 2048, 768
H, D = 12, 64
M = 2048
NCORES = 8
HPC = 3            # heads per core
NT = N // 128      # 16 query tiles
MT = M // 128      # 16 key tiles
MP = MT // 2       # 8 key-tile pairs
CT = DIM // 128    # 6 contraction tiles for qproj
QN = 512           # attention-unit query granularity (av psum = 1 bank)
NQ = N // QN       # 4 quarters
F32 = mybir.dt.float32
BF16 = mybir.dt.bfloat16
I16 = mybir.dt.int16

# Schraudolph fast-exp constants for the DVE offload path, in bf16
# bit-space: i16 = convert(s * A + B); bitcast(i16) as bf16 ~= exp(0.125*s),
# max rel err ~3%. A = 2^7 * 0.125 * log2(e); B = (127 - 0.0436) * 2^7
# (host-tuned minimax shift).
EXP_A = 23.083120654232846
EXP_B = 16250.4192
# m-tile pairs whose exp runs on the DVE instead of ScalarE
DVE_PAIRS = {2, 5}

_cached = {}
DEBUG_TAPS = False


def build_program():
    nc = bacc.Bacc("TRN2", target_bir_lowering=False, debug=False)
    xT_d = nc.dram_tensor("xT", [DIM, N], BF16, kind="ExternalInput")
    wq01_d = nc.dram_tensor("wq01", [128, CT, 128], BF16,
                            kind="ExternalInput")
    wq2_d = nc.dram_tensor("wq2", [128, CT, 128], BF16,
                           kind="ExternalInput")
    kT_d = nc.dram_tensor("kT", [128, HPC, MP, 128], BF16,
                          kind="ExternalInput")
    va_d = nc.dram_tensor("va", [128, HPC, MT, 128], BF16,
                          kind="ExternalInput")
    wp01_d = nc.dram_tensor("wp01", [128, DIM], BF16, kind="ExternalInput")
    wp2_d = nc.dram_tensor("wp2", [64, DIM], BF16, kind="ExternalInput")
    out_d = nc.dram_tensor("out", [N, DIM], F32, kind="ExternalOutput")
    if DEBUG_TAPS:
        qTdump_d = nc.dram_tensor("qTdump", [128, HPC, N], BF16,
                                  kind="ExternalOutput")
        otdump_d = nc.dram_tensor("otdump", [128, N], BF16,
                                  kind="ExternalOutput")
        ot2dump_d = nc.dram_tensor("ot2dump", [64, N], BF16,
                                   kind="ExternalOutput")

    with tile.TileContext(nc) as tc, ExitStack() as ctx:
        big = ctx.enter_context(tc.tile_pool(name="big", bufs=1))
        expp = ctx.enter_context(tc.tile_pool(name="expp", bufs=7))
        expi = ctx.enter_context(tc.tile_pool(name="expi", bufs=3))
        nrm = ctx.enter_context(tc.tile_pool(name="nrm", bufs=3))
        stg = ctx.enter_context(tc.tile_pool(name="stg", bufs=3))

        # ACT table warmup: a tiny exp at t~0 so the ~2.7us table load is
        # off the critical path of the first real exp
        wu = big.tile([128, 8], F32)
        nc.gpsimd.memset(wu[:], 0.0)
        wu2 = big.tile([128, 8], F32)
        nc.scalar.activation(wu2[:], wu[:], mybir.ActivationFunctionType.Exp)

        # persistent SBUF tensors; DMA order = consumption order: wq and
        # xT quarter 0 pace qproj(q0), then all k/v (units of quarter 0 run
        # through all 3 heads), then the later xT quarters and wp.
        # va_r (f32 copy for the DVE-exp AV path) is converted on the idle
        # gpsimd engine as each va head lands.
        wq01_t = big.tile([128, CT, 128], BF16)
        nc.sync.dma_start(wq01_t[:], wq01_d[:])
        wq2_t = big.tile([128, CT, 128], BF16)
        nc.sync.dma_start(wq2_t[:], wq2_d[:])
        xT_t = [big.tile([128, N], BF16, name=f"xT{c}", tag=f"xT{c}")
                for c in range(CT)]
        kT_t = big.tile([128, HPC, MP, 128], BF16)
        va_t = big.tile([128, HPC, MT, 128], BF16)

        def _dma_xq(q):
            for c in range(CT):
                nc.sync.dma_start(xT_t[c][:, q * QN:(q + 1) * QN],
                                  xT_d[c * 128:(c + 1) * 128,
                                       q * QN:(q + 1) * QN])

        def _dma_kv(h):
            nc.sync.dma_start(kT_t[:, h], kT_d[:, h])
            nc.sync.dma_start(va_t[:, h], va_d[:, h])

        _dma_xq(0)
        _dma_kv(0)
        _dma_kv(1)
        _dma_xq(1)
        _dma_kv(2)
        wp01_t = big.tile([128, DIM], BF16)
        nc.sync.dma_start(wp01_t[:], wp01_d[:])
        wp2_t = big.tile([64, DIM], BF16)
        nc.sync.dma_start(wp2_t[:], wp2_d[:])
        _dma_xq(2)
        _dma_xq(3)

        qT_t = big.tile([128, HPC, N], BF16)
        # proj contraction operands: heads 0|1 stacked on the partition dim,
        # head 2 separate
        outTn01_t = big.tile([128, N], BF16)
        outTn2_t = big.tile([64, N], BF16)

        # single instruction stream: per quarter, qproj (into the st psum
        # pool) then the 3 attention units; proj tiles of the previous
        # quarter interleave one per iteration. PSUM: st 2x2 banks +
        # av 2x1 + pj 2x1 + 2 spare.
        with tc.tile_pool(name="st_ps", bufs=2, space="PSUM") as st_ps, \
             tc.tile_pool(name="av_ps", bufs=2, space="PSUM") as av_ps, \
             tc.tile_pool(name="pj_ps", bufs=2, space="PSUM") as pj_ps:
            av_by_unit = {}
            pend = []
            proj_todo = []
            LAG = 2

            def _av(pd):
                (h, q), et0, et1, p, first, last = pd
                av = av_by_unit[(h, q)]
                nc.tensor.matmul(av[:], va_t[:, h, 2 * p], et0,
                                 start=first, stop=False)
                nc.tensor.matmul(av[:], va_t[:, h, 2 * p + 1], et1,
                                 start=False, stop=last)

            def _norm(unit):
                # denominator copied to a base-partition-0 SBUF tile first:
                # the custom-DVE recip misbehaves on HW when its input AP
                # sits at a partition offset (sim-only correct)
                h, q = unit
                av = av_by_unit.pop(unit)
                nsl = slice(q * QN, (q + 1) * QN)
                dn = nrm.tile([64, QN], F32, tag="dn", name="dn")
                nc.vector.tensor_copy(dn[:], av[64:128, :])
                rs = nrm.tile([64, QN], F32, tag="rs", name="rs")
                nc.vector.reciprocal_approx_fast(rs[:], dn[:])
                if h == 0:
                    dst = outTn01_t[0:64, nsl]
                elif h == 1:
                    dst = outTn01_t[64:128, nsl]
                else:
                    dst = outTn2_t[:, nsl]
                nc.vector.tensor_mul(dst, av[0:64, :], rs[:])

            qp_by = {}

            def _qproj_part(q, grp, part):
                # q projection for one (quarter, head-group), 2 c-tiles per
                # call so the PE bubble it injects into the attention stream
                # stays small; accumulates in a pj-pool buffer. grp 0 stacks
                # heads 0|1 in the stationary free dim (no duplication); the
                # copies fan the halves out into qT's duplicated layout.
                if part == 0:
                    qp_by[(q, grp)] = pj_ps.tile([128, 512], F32,
                                                 tag="pp", name="qp")
                qp = qp_by[(q, grp)]
                wq_t = wq01_t if grp == 0 else wq2_t
                for c in (2 * part, 2 * part + 1):
                    nc.tensor.matmul(
                        qp[:], wq_t[:, c],
                        xT_t[c][:, q * QN:(q + 1) * QN],
                        start=(c == 0), stop=(c == CT - 1),
                    )
                if part == 2:
                    nsl = slice(q * QN, (q + 1) * QN)
                    if grp == 0:
                        nc.vector.tensor_copy(qT_t[0:64, 0, nsl], qp[0:64])
                        nc.vector.tensor_copy(qT_t[64:128, 0, nsl], qp[0:64])
                        nc.vector.tensor_copy(qT_t[0:64, 1, nsl], qp[64:128])
                        nc.vector.tensor_copy(qT_t[64:128, 1, nsl],
                                              qp[64:128])
                    else:
                        nc.vector.tensor_copy(qT_t[:, 2, nsl], qp[:])
                    del qp_by[(q, grp)]

            def _proj_half(q, j, oc):
                # one 128-row n-tile x 384 out-cols; heads 0|1 via a single
                # 128-deep contraction, head 2 accumulated on top
                nn = (q * 4 + j) * 128
                osl = slice(oc * 384, (oc + 1) * 384)
                pp = pj_ps.tile([128, 512], F32, tag="pp", name="pp")
                nc.tensor.matmul(pp[:, 0:384], outTn01_t[:, nn:nn + 128],
                                 wp01_t[:, osl], start=True, stop=False)
                nc.tensor.matmul(pp[:, 0:384], outTn2_t[:, nn:nn + 128],
                                 wp2_t[:, osl], start=False, stop=True,
                                 tile_position=(0, 0))
                os_t = stg.tile([128, 384], F32, tag="os", name="os")
                if oc == 0:
                    nc.vector.tensor_copy(os_t[:], pp[:, 0:384])
                else:
                    nc.scalar.copy(os_t[:], pp[:, 0:384])
                nc.sync.dma_start(out_d[nn:nn + 128, osl], os_t[:])

            def _flush(limit):
                while len(pend) > limit:
                    pd = pend.pop(0)
                    _av(pd)
                    if pd[5]:
                        _norm(pd[0])
                        h, q = pd[0]
                        if h == HPC - 1:
                            proj_todo.extend(
                                (q, j, oc) for j in range(4) for oc in range(2))

            for grp in range(2):
                for part in range(3):
                    _qproj_part(0, grp, part)
            iters = [(h, q, p)
                     for q in range(NQ) for h in range(HPC) for p in range(MP)]
            for idx, (h, q, p) in enumerate(iters):
                unit = (h, q)
                if unit not in av_by_unit:
                    av_by_unit[unit] = av_ps.tile(
                        [128, QN], F32, tag="av", name="av")
                n0 = q * QN
                st = st_ps.tile([128, 2, 512], F32, tag="st", name="st")
                nc.tensor.matmul(
                    st[:, 0], kT_t[0:64, h, p], qT_t[0:64, h, n0:n0 + QN],
                    start=True, stop=True, tile_position=(0, 0),
                )
                nc.tensor.matmul(
                    st[:, 1], kT_t[64:128, h, p], qT_t[64:128, h, n0:n0 + QN],
                    start=True, stop=True, tile_position=(64, 0),
                )
                # prefill: don't issue AVs behind the first unit's S^T/exp
                # stream, so it isn't queue-blocked on the va DMA; drain the
                # backlog one entry per iteration afterwards
                _flush(max(LAG - 1, 15 - idx))
                if p in DVE_PAIRS:
                    eti = expi.tile([128, 2, 512], I16, tag="eti", name="eti")
                    nc.vector.tensor_scalar(
                        eti[:], st[:], EXP_A, EXP_B,
                        mybir.AluOpType.mult, mybir.AluOpType.add)
                    et0 = eti[:, 0].bitcast(BF16)
                    et1 = eti[:, 1].bitcast(BF16)
                else:
                    et = expp.tile([128, 2, 512], BF16, tag="et", name="et")
                    nc.scalar.activation(
                        et[:], st[:], mybir.ActivationFunctionType.Exp,
                        scale=float(D) ** -0.5,
                    )
                    et0, et1 = et[:, 0], et[:, 1]
                pend.append((unit, et0, et1, p, p == 0, p == MP - 1))
                if q + 1 < NQ and p >= MP - 3 and h < 2:
                    _qproj_part(q + 1, h, p - (MP - 3))
                if proj_todo:
                    _proj_half(*proj_todo.pop(0))
            _flush(0)
            while proj_todo:
                _proj_half(*proj_todo.pop(0))
            if DEBUG_TAPS:
                nc.sync.dma_start(qTdump_d[:], qT_t[:])
                nc.sync.dma_start(otdump_d[:], outTn01_t[:])
                nc.sync.dma_start(ot2dump_d[:], outTn2_t[:])

    nc.compile()
    return nc


def build_in_maps(x, k, v, w_qkv, w_proj):
    x = np.asarray(x, dtype=np.float32)
    k = np.asarray(k, dtype=np.float32)
    v = np.asarray(v, dtype=np.float32)
    wqT = np.ascontiguousarray(np.asarray(w_qkv, np.float32).T)   # [C, 768]
    wpT = np.ascontiguousarray(np.asarray(w_proj, np.float32).T)  # [768, 768]
    bf = ml_dtypes.bfloat16

    in_maps = []
    for core in range(NCORES):
        b = core // 4
        hs = [3 * (core % 4) + i for i in range(HPC)]
        xT = np.ascontiguousarray(x[b].T.astype(bf))
        # wq01 [128, CT, 128]: heads 0|1 stacked in the output columns;
        # wq2: head 2 duplicated -> qT rows 64:128 == rows 0:64
        b0 = wqT[:, 64 * hs[0]:64 * hs[0] + 64]
        b1 = wqT[:, 64 * hs[1]:64 * hs[1] + 64]
        b2 = wqT[:, 64 * hs[2]:64 * hs[2] + 64]
        wq01 = (np.concatenate([b0, b1], axis=1)
                .reshape(CT, 128, 128).transpose(1, 0, 2).astype(bf))
        wq2 = (np.concatenate([b2, b2], axis=1)
               .reshape(CT, 128, 128).transpose(1, 0, 2).astype(bf))
        # kT [128, HPC, MP, 128]: rows 0:64 = head-dim of even m-tile,
        # rows 64:128 = head-dim of odd m-tile of each pair
        kb = k[b, hs].astype(bf)                            # [3, M, D]
        kT = np.empty((128, HPC, MP, 128), dtype=bf)
        for hi in range(HPC):
            for p in range(MP):
                kT[0:64, hi, p, :] = kb[hi, 256 * p:256 * p + 128, :].T
                kT[64:128, hi, p, :] = kb[hi, 256 * p + 128:256 * p + 256, :].T
        # va [128, HPC, MT, 128]: [v | ones]; partition = key-within-tile
        va = np.ones((128, HPC, MT, 128), dtype=bf)
        va[:, :, :, :D] = (
            v[b, hs].reshape(HPC, MT, 128, D).transpose(2, 0, 1, 3).astype(bf))
        # wp01 [128, DIM]: heads 0|1 stacked on partitions; wp2 [64, DIM]
        wp01 = np.empty((128, DIM), dtype=bf)
        wp01[0:64] = wpT[64 * hs[0]:64 * hs[0] + 64, :].astype(bf)
        wp01[64:128] = wpT[64 * hs[1]:64 * hs[1] + 64, :].astype(bf)
        wp2 = np.ascontiguousarray(
            wpT[64 * hs[2]:64 * hs[2] + 64, :].astype(bf))
        in_maps.append({"xT": xT,
                        "wq01": np.ascontiguousarray(wq01),
                        "wq2": np.ascontiguousarray(wq2),
                        "kT": np.ascontiguousarray(kT),
                        "va": np.ascontiguousarray(va),
                        "wp01": wp01, "wp2": wp2})
    return in_maps


def kernel(x, k, v, w_qkv, w_proj, b_proj):
    b_proj = np.asarray(b_proj, dtype=np.float32)

    if "nc" not in _cached:
        _cached["nc"] = build_program()
    nc = _cached["nc"]

    in_maps = build_in_maps(x, k, v, w_qkv, w_proj)
    res = run_bass_kernel_spmd(nc, in_maps, core_ids=list(range(NCORES)))

    out = np.empty((B, N, DIM), dtype=np.float32)
    for b in range(B):
        acc = np.zeros((N, DIM), dtype=np.float64)
        for core in range(4 * b, 4 * b + 4):
            acc += res.results[core]["out"]
        out[b] = (acc + b_proj).astype(np.float32)
    return out


# revision 37
# speedup vs baseline: 1.2798x; 1.0037x over previous
"""Multi-head attention (GAttention) on 8 trn2 NeuronCores.

Reference computation (per batch b):
    q = x @ w_qkv.T            -> [N, 768], heads of 64
    attn = softmax(q k^T / 8)  -> per head [N, M]
    out_h = attn @ v           -> [N, 64]
    out = concat(out_h) @ w_proj.T + b_proj

Sharding: 24 (b, head) units over 8 cores -> each core gets one batch b and
3 heads. Each core computes its heads' attention plus its partial
projection sum [N, 768]; host adds the 4 partials per batch + bias.

The kernel is ScalarE-bound: softmax exp is ACT-only at 1 elem/lane/cycle
(1.2 GHz), 12.6M elems/core ~= 96 us. Everything else is scheduled around
keeping the ACT exp stream dense from ~9 us on:
  1. qproj (bf16) paced by the xT DMA stream, n-half outer so head-0
     queries finish early; wq has the head slice duplicated so qT rows
     64:128 copy rows 0:64 (feeds the row-packed S^T matmuls).
  2. attention in 12 (head, n-quarter) units; per key m-tile PAIR:
       S^T = k q^T   (PE row groups 0/64 concurrently) -> PSUM [128,2,512]
       expT = exp(0.125 S^T) -> SBUF bf16 (ACT, fused scale)
       AV: av[128,512] += va^T expT  (va = [v | ones]; rows 64:128 of av
           hold the softmax denominator), issued LAG iterations behind.
  3. normalize: outTn[0:64] = av[0:64] * recip_approx(av[64:128])
  4. proj (bf16): per 128-row n-tile, 3 heads accumulate in PSUM; output
     DMA'd straight from PSUM to DRAM f32. Proj tiles are interleaved into
     the following attention unit so only the last quarter is tail work.
"""
import numpy as np
import ml_dtypes
from contextlib import ExitStack

import concourse.bass as bass
import concourse.mybir as mybir
import concourse.tile as tile
from concourse import bacc
from concourse.bass_utils import run_bass_kernel_spmd

B, N, DIM = 2, 2048, 768
H, D = 12, 64
M = 2048
NCORES = 8
HPC = 3            # heads per core
NT = N // 128      # 16 query tiles
MT = M // 128      # 16 key tiles
MP = MT // 2       # 8 key-tile pairs
CT = DIM // 128    # 6 contraction tiles for qproj
QN = 512           # attention-unit query granularity (av psum = 1 bank)
NQ = N // QN       # 4 quarters
F32 = mybir.dt.float32
BF16 = mybir.dt.bfloat16
I16 = mybir.dt.int16

# Schraudolph fast-exp constants for the DVE offload path, in bf16
# bit-space: i16 = convert(s * A + B); bitcast(i16) as bf16 ~= exp(0.125*s),
# max rel err ~3%. A = 2^7 * 0.125 * log2(e); B = (127 - 0.0436) * 2^7
# (host-tuned minimax shift).
EXP_A = 23.083120654232846
EXP_B = 16250.4192
# m-tile pairs whose exp runs on the DVE instead of ScalarE
DVE_PAIRS = {2, 5}

_cached = {}
DEBUG_TAPS = False


def build_program():
    nc = bacc.Bacc("TRN2", target_bir_lowering=False, debug=False)
    xT_d = nc.dram_tensor("xT", [DIM, N], BF16, kind="ExternalInput")
    wq01_d = nc.dram_tensor("wq01", [128, CT, 128], BF16,
                            kind="ExternalInput")
    wq2_d = nc.dram_tensor("wq2", [128, CT, 128], BF16,
                           kind="ExternalInput")
    kT_d = nc.dram_tensor("kT", [128, HPC, MP, 128], BF16,
                          kind="ExternalInput")
    va_d = nc.dram_tensor("va", [128, HPC, MT, 128], BF16,
                          kind="ExternalInput")
    wp01_d = nc.dram_tensor("wp01", [128, DIM], BF16, kind="ExternalInput")
    wp2_d = nc.dram_tensor("wp2", [64, DIM], BF16, kind="ExternalInput")
    out_d = nc.dram_tensor("out", [N, DIM], F32, kind="ExternalOutput")
    if DEBUG_TAPS:
        qTdump_d = nc.dram_tensor("qTdump", [128, HPC, N], BF16,
                                  kind="ExternalOutput")
        otdump_d = nc.dram_tensor("otdump", [128, N], BF16,
                                  kind="ExternalOutput")
        ot2dump_d = nc.dram_tensor("ot2dump", [64, N], BF16,
                                   kind="ExternalOutput")

    with tile.TileContext(nc) as tc, ExitStack() as ctx:
        big = ctx.enter_context(tc.tile_pool(name="big", bufs=1))
        expp = ctx.enter_context(tc.tile_pool(name="expp", bufs=7))
        expi = ctx.enter_context(tc.tile_pool(name="expi", bufs=3))
        nrm = ctx.enter_context(tc.tile_pool(name="nrm", bufs=3))
        stg = ctx.enter_context(tc.tile_pool(name="stg", bufs=3))

        # ACT table warmup: a tiny exp at t~0 so the ~2.7us table load is
        # off the critical path of the first real exp
        wu = big.tile([128, 8], F32)
        nc.gpsimd.memset(wu[:], 0.0)
        wu2 = big.tile([128, 8], F32)
        nc.scalar.activation(wu2[:], wu[:], mybir.ActivationFunctionType.Exp)

        # persistent SBUF tensors; DMA order = consumption order: wq and
        # xT quarter 0 pace qproj(q0), then all k/v (units of quarter 0 run
        # through all 3 heads), then the later xT quarters and wp.
        # va_r (f32 copy for the DVE-exp AV path) is converted on the idle
        # gpsimd engine as each va head lands.
        wq01_t = big.tile([128, CT, 128], BF16)
        nc.sync.dma_start(wq01_t[:], wq01_d[:])
        wq2_t = big.tile([128, CT, 128], BF16)
        nc.sync.dma_start(wq2_t[:], wq2_d[:])
        xT_t = [big.tile([128, N], BF16, name=f"xT{c}", tag=f"xT{c}")
                for c in range(CT)]
        kT_t = big.tile([128, HPC, MP, 128], BF16)
        va_t = big.tile([128, HPC, MT, 128], BF16)

        def _dma_xq(q):
            for c in range(CT):
                nc.sync.dma_start(xT_t[c][:, q * QN:(q + 1) * QN],
                                  xT_d[c * 128:(c + 1) * 128,
                                       q * QN:(q + 1) * QN])

        def _dma_kv(h):
            nc.sync.dma_start(kT_t[:, h], kT_d[:, h])
            nc.sync.dma_start(va_t[:, h], va_d[:, h])

        _dma_xq(0)
        _dma_kv(0)
        _dma_kv(1)
        _dma_xq(1)
        _dma_kv(2)
        wp01_t = big.tile([128, DIM], BF16)
        nc.sync.dma_start(wp01_t[:], wp01_d[:])
        wp2_t = big.tile([64, DIM], BF16)
        nc.sync.dma_start(wp2_t[:], wp2_d[:])
        _dma_xq(2)
        _dma_xq(3)

        qT_t = big.tile([128, HPC, N], BF16)
        # proj contraction operands: heads 0|1 stacked on the partition dim,
        # head 2 separate
        outTn01_t = big.tile([128, N], BF16)
        outTn2_t = big.tile([64, N], BF16)

        # single instruction stream: per quarter, qproj (into the st psum
        # pool) then the 3 attention units; proj tiles of the previous
        # quarter interleave one per iteration. PSUM: st 2x2 banks +
        # av 2x1 + pj 2x1 + 2 spare.
        with tc.tile_pool(name="st_ps", bufs=2, space="PSUM") as st_ps, \
             tc.tile_pool(name="av_ps", bufs=2, space="PSUM") as av_ps, \
             tc.tile_pool(name="pj_ps", bufs=2, space="PSUM") as pj_ps:
            av_by_unit = {}
            pend = []
            proj_todo = []
            LAG = 2

            def _av(pd):
                (h, q), et0, et1, p, first, last = pd
                av = av_by_unit[(h, q)]
                nc.tensor.matmul(av[:], va_t[:, h, 2 * p], et0,
                                 start=first, stop=False)
                nc.tensor.matmul(av[:], va_t[:, h, 2 * p + 1], et1,
                                 start=False, stop=last)

            def _norm(unit):
                # denominator copied to a base-partition-0 SBUF tile first:
                # the custom-DVE recip misbehaves on HW when its input AP
                # sits at a partition offset (sim-only correct)
                h, q = unit
                av = av_by_unit.pop(unit)
                nsl = slice(q * QN, (q + 1) * QN)
                dn = nrm.tile([64, QN], F32, tag="dn", name="dn")
                nc.vector.tensor_copy(dn[:], av[64:128, :])
                rs = nrm.tile([64, QN], F32, tag="rs", name="rs")
                nc.vector.reciprocal_approx_fast(rs[:], dn[:])
                if h == 0:
                    dst = outTn01_t[0:64, nsl]
                elif h == 1:
                    dst = outTn01_t[64:128, nsl]
                else:
                    dst = outTn2_t[:, nsl]
                nc.vector.tensor_mul(dst, av[0:64, :], rs[:])

            qp_by = {}

            def _qproj_part(q, grp, part):
                # q projection for one (quarter, head-group), 2 c-tiles per
                # call so the PE bubble it injects into the attention stream
                # stays small; accumulates in a pj-pool buffer. grp 0 stacks
                # heads 0|1 in the stationary free dim (no duplication); the
                # copies fan the halves out into qT's duplicated layout.
                if part == 0:
                    qp_by[(q, grp)] = pj_ps.tile([128, 512], F32,
                                                 tag="pp", name="qp")
                qp = qp_by[(q, grp)]
                wq_t = wq01_t if grp == 0 else wq2_t
                for c in (2 * part, 2 * part + 1):
                    nc.tensor.matmul(
                        qp[:], wq_t[:, c],
                        xT_t[c][:, q * QN:(q + 1) * QN],
                        start=(c == 0), stop=(c == CT - 1),
                    )
                if part == 2:
                    nsl = slice(q * QN, (q + 1) * QN)
                    if grp == 0:
                        nc.vector.tensor_copy(qT_t[0:64, 0, nsl], qp[0:64])
                        nc.vector.tensor_copy(qT_t[64:128, 0, nsl], qp[0:64])
                        nc.vector.tensor_copy(qT_t[0:64, 1, nsl], qp[64:128])
                        nc.vector.tensor_copy(qT_t[64:128, 1, nsl],
                                              qp[64:128])
                    else:
                        nc.vector.tensor_copy(qT_t[:, 2, nsl], qp[:])
                    del qp_by[(q, grp)]

            def _proj_half(q, j, oc):
                # one 128-row n-tile x 384 out-cols; heads 0|1 via a single
                # 128-deep contraction, head 2 accumulated on top
                nn = (q * 4 + j) * 128
                osl = slice(oc * 384, (oc + 1) * 384)
                pp = pj_ps.tile([128, 512], F32, tag="pp", name="pp")
                nc.tensor.matmul(pp[:, 0:384], outTn01_t[:, nn:nn + 128],
                                 wp01_t[:, osl], start=True, stop=False)
                nc.tensor.matmul(pp[:, 0:384], outTn2_t[:, nn:nn + 128],
                                 wp2_t[:, osl], start=False, stop=True,
                                 tile_position=(0, 0))
                os_t = stg.tile([128, 384], F32, tag="os", name="os")
                if oc == 0:
                    nc.vector.tensor_copy(os_t[:], pp[:, 0:384])
                else:
                    nc.scalar.copy(os_t[:], pp[:, 0:384])
                nc.sync.dma_start(out_d[nn:nn + 128, osl], os_t[:])

            def _flush(limit):
                while len(pend) > limit:
                    pd = pend.pop(0)
                    _av(pd)
                    if pd[5]:
                        _norm(pd[0])
                        h, q = pd[0]
                        if h == HPC - 1:
                            proj_todo.extend(
                                (q, j, oc) for j in range(4) for oc in range(2))

            for grp in range(2):
                for part in range(3):
                    _qproj_part(0, grp, part)
            iters = [(h, q, p)
                     for q in range(NQ) for h in range(HPC) for p in range(MP)]
            for idx, (h, q, p) in enumerate(iters):
                unit = (h, q)
                if unit not in av_by_unit:
                    av_by_unit[unit] = av_ps.tile(
                        [128, QN], F32, tag="av", name="av")
                n0 = q * QN
                st = st_ps.tile([128, 2, 512], F32, tag="st", name="st")
                nc.tensor.matmul(
                    st[:, 0], kT_t[0:64, h, p], qT_t[0:64, h, n0:n0 + QN],
                    start=True, stop=True, tile_position=(0, 0),
                )
                nc.tensor.matmul(
                    st[:, 1], kT_t[64:128, h, p], qT_t[64:128, h, n0:n0 + QN],
                    start=True, stop=True, tile_position=(64, 0),
                )
                # prefill: don't issue AVs behind the first unit's S^T/exp
                # stream, so it isn't queue-blocked on the va DMA; drain the
                # backlog one entry per iteration afterwards
                _flush(max(LAG - 1, 15 - idx))
                if p in DVE_PAIRS:
                    eti = expi.tile([128, 2, 512], I16, tag="eti", name="eti")
                    nc.vector.tensor_scalar(
                        eti[:], st[:], EXP_A, EXP_B,
                        mybir.AluOpType.mult, mybir.AluOpType.add)
                    et0 = eti[:, 0].bitcast(BF16)
                    et1 = eti[:, 1].bitcast(BF16)
                else:
                    et = expp.tile([128, 2, 512], BF16, tag="et", name="et")
                    nc.scalar.activation(
                        et[:], st[:], mybir.ActivationFunctionType.Exp,
                        scale=float(D) ** -0.5,
                    )
                    et0, et1 = et[:, 0], et[:, 1]
                pend.append((unit, et0, et1, p, p == 0, p == MP - 1))
                if q + 1 < NQ and p >= MP - 3 and h < 2:
                    _qproj_part(q + 1, h, p - (MP - 3))
                if proj_todo:
                    _proj_half(*proj_todo.pop(0))
            _flush(0)
            while proj_todo:
                _proj_half(*proj_todo.pop(0))
            if DEBUG_TAPS:
                nc.sync.dma_start(qTdump_d[:], qT_t[:])
                nc.sync.dma_start(otdump_d[:], outTn01_t[:])
                nc.sync.dma_start(ot2dump_d[:], outTn2_t[:])

    nc.compile()
    return nc


def build_in_maps(x, k, v, w_qkv, w_proj):
    x = np.asarray(x, dtype=np.float32)
    k = np.asarray(k, dtype=np.float32)
    v = np.asarray(v, dtype=np.float32)
    wqT = np.ascontiguousarray(np.asarray(w_qkv, np.float32).T)   # [C, 768]
    wpT = np.ascontiguousarray(np.asarray(w_proj, np.float32).T)  # [768, 768]
    bf = ml_dtypes.bfloat16

    in_maps = []
    for core in range(NCORES):
        b = core // 4
        hs = [3 * (core % 4) + i for i in range(HPC)]
        xT = np.ascontiguousarray(x[b].T.astype(bf))
        # wq01 [128, CT, 128]: heads 0|1 stacked in the output columns;
        # wq2: head 2 duplicated -> qT rows 64:128 == rows 0:64
        b0 = wqT[:, 64 * hs[0]:64 * hs[0] + 64]
        b1 = wqT[:, 64 * hs[1]:64 * hs[1] + 64]
        b2 = wqT[:, 64 * hs[2]:64 * hs[2] + 64]
        wq01 = (np.concatenate([b0, b1], axis=1)
                .reshape(CT, 128, 128).transpose(1, 0, 2).astype(bf))
        wq2 = (np.concatenate([b2, b2], axis=1)
               .reshape(CT, 128, 128).transpose(1, 0, 2).astype(bf))
        # kT [128, HPC, MP, 128]: rows 0:64 = head-dim of even m-tile,
        # rows 64:128 = head-dim of odd m-tile of each pair
        kb = k[b, hs].astype(bf)                            # [3, M, D]
        kT = np.empty((128, HPC, MP, 128), dtype=bf)
        for hi in range(HPC):
            for p in range(MP):
                kT[0:64, hi, p, :] = kb[hi, 256 * p:256 * p + 128, :].T
                kT[64:128, hi, p, :] = kb[hi, 256 * p + 128:256 * p + 256, :].T
        # va [128, HPC, MT, 128]: [v | ones]; partition = key-within-tile
        va = np.ones((128, HPC, MT, 128), dtype=bf)
        va[:, :, :, :D] = (
            v[b, hs].reshape(HPC, MT, 128, D).transpose(2, 0, 1, 3).astype(bf))
        # wp01 [128, DIM]: heads 0|1 stacked on partitions; wp2 [64, DIM]
        wp01 = np.empty((128, DIM), dtype=bf)
        wp01[0:64] = wpT[64 * hs[0]:64 * hs[0] + 64, :].astype(bf)
        wp01[64:128] = wpT[64 * hs[1]:64 * hs[1] + 64, :].astype(bf)
        wp2 = np.ascontiguousarray(
            wpT[64 * hs[2]:64 * hs[2] + 64, :].astype(bf))
        in_maps.append({"xT": xT,
                        "wq01": np.ascontiguousarray(wq01),
                        "wq2": np.ascontiguousarray(wq2),
                        "kT": np.ascontiguousarray(kT),
                        "va": np.ascontiguousarray(va),
                        "wp01": wp01, "wp2": wp2})
    return in_maps


def kernel(x, k, v, w_qkv, w_proj, b_proj):
    b_proj = np.asarray(b_proj, dtype=np.float32)

    if "nc" not in _cached:
        _cached["nc"] = build_program()
    nc = _cached["nc"]

    in_maps = build_in_maps(x, k, v, w_qkv, w_proj)
    res = run_bass_kernel_spmd(nc, in_maps, core_ids=list(range(NCORES)))

    out = np.empty((B, N, DIM), dtype=np.float32)
    for b in range(B):
        acc = np.zeros((N, DIM), dtype=np.float64)
        for core in range(4 * b, 4 * b + 4):
            acc += res.results[core]["out"]
        out[b] = (acc + b_proj).astype(np.float32)
    return out
